# revision 35
# baseline (speedup 1.0000x reference)
"""DGConv (GCN diffusion, K=2 steps) on 8 Trainium2 NeuronCores.

Math (equivalent reformulation of the reference):
  deg_i  = indegree_i + 1 (self loop),  dinv = deg^-1/2
  z      = dinv * x            (per-edge weight dinv_s*dinv_d folds away)
  S(z)_i = sum_{(j->i) in E} z_j         (unweighted segment sum, no self loops)
  step:  z' = (1-d)*z + d*(1/deg)*(S(z) + z),   d = T/K
  out    = (sqrt(deg) * z_K) @ W + b
         = [S2*(d*dinv) + z1*((1-d)sqrt(deg)+d*dinv)] @ W + b
           (the Linear commutes with all row-wise scaling and with S)

Distribution: dst-node ranges sharded 8 ways (SH=12544 nodes/core).  Each
core gathers z[src] for its edges from a replicated HBM table via dma_gather
(int16 indices -> 4 quarter windows of the table), segment-sums with one-hot
matmuls on the PE (128-dst-node blocks, PSUM accumulation), and AllGathers
the updated z shard between the two propagation steps.

Table rows are permuted partition-major (node (c,b,p) -> row c*SH+p*NB+b) so
internal DMAs are contiguous per partition; the host un-permutes at the end.
Table rows are 128 fp16 (=256B, dma_gather granularity); cols 64:128 unused.
"""

import numpy as np

# ---------------------------------------------------------------- constants
N = 100000
D = 64
T = 5.27
KSTEPS = 2
DELTA = T / KSTEPS

DEFAULT_CFG = dict(
    N=100000,
    NCORES=8,
    PB=128,        # dst nodes per block (one-hot width)
    NB=100,        # blocks per core (1.7% slack under the 4-chunk cap)
    GB=2,          # blocks per gather group
    NQ=4,          # int16 index windows
    BALANCE=True,  # greedy per-(block,quarter) edge-count balancing
    STRIPE_Q=True, # lane-stripe quarters -> chunked (per-quarter) AllGather
)

_CACHE = {}


def _finish_cfg(cfg):
    cfg = dict(cfg)
    cfg["SH"] = cfg["PB"] * cfg["NB"]
    cfg["NPAD"] = cfg["SH"] * cfg["NCORES"]
    assert cfg["NPAD"] % cfg["NQ"] == 0
    cfg["QROWS"] = cfg["NPAD"] // cfg["NQ"]
    assert cfg["QROWS"] <= 32768
    return cfg


def _bass_modules():
    import sys
    if "/opt/trn_rl_repo" not in sys.path:
        sys.path.insert(0, "/opt/trn_rl_repo")
    import concourse.bacc as bacc
    import concourse.tile as tile
    import concourse.mybir as mybir
    from concourse import bass
    from concourse.bass_utils import run_bass_kernel_spmd
    return bacc, tile, mybir, bass, run_bass_kernel_spmd


# ---------------------------------------------------------------- host prep
def _core_of_nodes(cfg):
    """Node -> core. Real nodes split evenly (N/NCORES each); pad nodes
    (N..NPAD) distributed evenly so no core carries extra real load."""
    n, ncores, npad = cfg["N"], cfg["NCORES"], cfg["NPAD"]
    real = n // ncores
    pads = (npad - n) // ncores
    core_of = np.empty(npad, np.int64)
    core_of[:n] = np.minimum(np.arange(n) // real, ncores - 1)
    core_of[n:] = np.arange(npad - n) // pads
    return core_of


def _balanced_assignment(dst, src, core_of, cfg):
    """Per core, pack its SH nodes into NB blocks of PB nodes, balancing the
    per-(block, src-quarter) edge counts (greedy vector LPT with soft cap).
    A node's gather quarter depends only on its core (core table rows share
    qrows windows), so packing cannot disturb quarter labels.
    Returns node_of_slot [NPAD]: slot c*SH + b*PB + p -> node id."""
    ncores, pb, sh, nb = cfg["NCORES"], cfg["PB"], cfg["SH"], cfg["NB"]
    npad, nq, qrows = cfg["NPAD"], cfg["NQ"], cfg["QROWS"]
    q_src = core_of[src] // (qrows // sh)      # quarter = f(core of src)
    dq = np.bincount(dst * nq + q_src, minlength=npad * nq).reshape(npad, nq)
    # global target chunks per (block, quarter); placements below target*PB
    # are free (the cross-core max will be at target anyway)
    tgt = max(1, -(-dq.sum() // (ncores * nb * nq * pb)))
    cap = tgt * pb
    node_of_slot = np.empty(npad, np.int64)
    BIG = 1 << 22
    bidx = np.arange(nb)
    for c in range(ncores):
        cnodes = np.where(core_of == c)[0]
        assert cnodes.shape[0] == sh
        v = dq[cnodes].astype(np.int64)                   # [SH, NQ]
        order = np.argsort(-v.sum(axis=1), kind="stable")
        sums = np.zeros((nb, nq), np.int64)
        cnts = np.zeros(nb, np.int64)
        binL = np.empty(sh, np.int64)                      # node -> block
        for nloc in order:
            vi = v[nloc]
            cand = sums + vi                               # [NB, NQ]
            # cost = chunk-count increase ABOVE the global target
            dch = (np.maximum((cand + pb - 1) // pb, tgt)
                   - np.maximum((sums + pb - 1) // pb, tgt)).sum(axis=1)
            # steer overflow to the same (low) block index on every core
            over = (cand > cap).any(axis=1)
            score = dch * BIG + over * bidx * 2048 + cand.max(axis=1)
            score[cnts >= pb] = 1 << 60
            bsel = int(np.argmin(score))
            sums[bsel] += vi
            binL[nloc] = bsel
            cnts[bsel] += 1
        lane = np.zeros(nb, np.int64)
        slots = np.empty(sh, np.int64)
        for nloc in range(sh):
            b = binL[nloc]
            slots[nloc] = b * pb + lane[b]
            lane[b] += 1
        node_of_slot[c * sh + slots] = cnodes
    return node_of_slot


def _stripe_lanes(dst, src, core_of, node_of_slot, cfg):
    """Reassign lanes within each (core, block) so per-(dst-core, dst-block,
    stripe) edge counts stay under the global chunk cap.  A node's lane
    stripe (lane//32) is the gather quarter of all its OUT-edges; greedy
    over nodes by out-degree, 32-per-stripe quota per block."""
    ncores, pb, sh, nb = cfg["NCORES"], cfg["PB"], cfg["SH"], cfg["NB"]
    nq, npad = cfg["NQ"], cfg["NPAD"]
    st = pb // nq
    E = dst.shape[0]
    slot_of_node = np.empty(npad, np.int64)
    slot_of_node[node_of_slot] = np.arange(npad)
    b_of = (slot_of_node % sh) // pb
    cell_e = core_of[dst] * nb + b_of[dst]
    eorder = np.argsort(src, kind="stable")
    src_s = src[eorder]
    cell_s = cell_e[eorder]
    starts = np.searchsorted(src_s, np.arange(npad + 1))
    outdeg = np.diff(starts)
    tgt = max(1, -(-E // (ncores * nb * nq * pb)))
    cap = tgt * pb
    cnt = np.zeros((ncores * nb, nq), np.int32)
    quota = np.full((ncores * nb, nq), st, np.int32)
    lane_q = np.empty(npad, np.int8)
    BIG = np.float64(1e18)
    for n in np.argsort(-outdeg, kind="stable"):
        qb = core_of[n] * nb + b_of[n]
        s0, s1 = starts[n], starts[n + 1]
        if s1 > s0:
            u, m = np.unique(cell_s[s0:s1], return_counts=True)
            base = cnt[u]                                    # [k, nq]
            cand = base + m[:, None]
            over = (np.maximum(cand - cap, 0)
                    - np.maximum(base - cap, 0)).sum(axis=0)
            score = over * 1e6 + cand.max(axis=0)
        else:
            score = np.zeros(nq)
        score = np.where(quota[qb] > 0, score, BIG)
        q = int(np.argmin(score))
        lane_q[n] = q
        quota[qb, q] -= 1
        if s1 > s0:
            np.add.at(cnt, (cell_s[s0:s1], q), 1)
    # lanes: stripe-major fill within each (core, block)
    keys = (core_of * nb + b_of) * nq + lane_q
    order2 = np.lexsort((np.arange(npad), keys))
    ks = keys[order2]
    first = np.ones(npad, bool)
    first[1:] = ks[1:] != ks[:-1]
    gs = np.maximum.accumulate(np.where(first, np.arange(npad), 0))
    rank = np.arange(npad) - gs
    c2, b2 = core_of[order2], b_of[order2]
    lane2 = lane_q[order2] * st + rank
    out = np.empty(npad, np.int64)
    out[c2 * sh + b2 * pb + lane2] = order2
    return out


def host_prep(x, edge_index, cfg):
    """Per-core input arrays + static chunk layout (shared across cores)."""
    n, ncores, pb = cfg["N"], cfg["NCORES"], cfg["PB"]
    sh, nb, npad = cfg["SH"], cfg["NB"], cfg["NPAD"]
    nq, qrows, gb = cfg["NQ"], cfg["QROWS"], cfg["GB"]

    src = np.asarray(edge_index[0], np.int64)
    dst = np.asarray(edge_index[1], np.int64)
    E = src.shape[0]

    deg = 1.0 + np.bincount(dst, minlength=npad)[:npad].astype(np.float32)

    if cfg.get("BALANCE"):
        core_of = _core_of_nodes(cfg)
        node_of_slot = _balanced_assignment(dst, src, core_of, cfg)
    else:
        core_of = np.arange(npad, dtype=np.int64) // sh
        node_of_slot = np.arange(npad, dtype=np.int64)
    if cfg.get("STRIPE_Q"):
        node_of_slot = _stripe_lanes(dst, src, core_of, node_of_slot, cfg)
    # inverse: node -> (block, lane) within its core
    slot_of_node = np.empty(npad, np.int64)
    slot_of_node[node_of_slot] = np.arange(npad)
    b_of = (slot_of_node % sh) // pb
    p_of = (slot_of_node % sh) % pb

    c_e = core_of[dst]
    b_e = b_of[dst]
    dloc_e = p_of[dst].astype(np.int32)
    if cfg.get("STRIPE_Q"):
        # stripe quarters: q = lane//(PB/NQ); table row
        #   q*(NPAD/NQ) + c*(SH/NQ) + (p % (PB/NQ))*NB + b
        # so quarter q of the table is exactly what chunk-AllGather q
        # delivers (each core sends lanes [32q:32q+32) of its z shard).
        st = pb // nq
        row_of = ((p_of // st) * (npad // nq) + core_of * (sh // nq)
                  + (p_of % st) * nb + b_of)          # node id -> table row
    else:
        # permuted table row: (c2, b2, p2) -> c2*SH + p2*NB + b2
        row_of = core_of * sh + p_of * nb + b_of
    pr_src = row_of[src]
    q_e = (pr_src // qrows).astype(np.int32)
    qi_e = (pr_src % qrows).astype(np.int32)

    # chunk budget per (block, quarter): max over cores
    key = ((c_e * nb + b_e) * nq + q_e).astype(np.int64)
    cnt = np.bincount(key, minlength=ncores * nb * nq).reshape(ncores, nb, nq)
    Cbq = -(-cnt.max(axis=0) // pb)              # [NB, NQ]

    groups = [range(g0, min(g0 + gb, nb)) for g0 in range(0, nb, gb)]
    col = 0
    chunk_col0 = np.zeros((nb, nq), np.int64)
    calls = []                                   # (gi, q, col0, ncols)
    group_col0 = []
    for gi, blocks in enumerate(groups):
        group_col0.append(col)
        for q in range(nq):
            start = col
            for bb in blocks:
                chunk_col0[bb, q] = col
                col += Cbq[bb, q]
            if col > start:
                calls.append((gi, q, start, col - start))
    totch = int(col)
    totslot = totch * pb
    group_col0.append(totch)

    # slot assignment: edges sorted by (c,b,q); rank within each segment
    order = np.lexsort((q_e, b_e, c_e))
    key_s = key[order]
    first = np.ones(E, bool)
    first[1:] = key_s[1:] != key_s[:-1]
    seg_start = np.maximum.accumulate(np.where(first, np.arange(E), 0))
    rank = np.arange(E) - seg_start
    slot = chunk_col0[b_e[order], q_e[order]] * pb + rank

    idx_all = np.zeros((ncores, totslot), np.int16)
    flat = c_e[order] * totslot + slot
    idx_all.reshape(-1)[flat] = qi_e[order].astype(np.int16)

    # dst-local table in BLOCK-MAJOR chunk order (for fused one-hot builds):
    # block b's chunks occupy consecutive columns dcol0[b] + qoff[b,q] + k
    Cb = Cbq.sum(axis=1)                      # chunks per block
    dcol0 = np.concatenate([[0], np.cumsum(Cb)])[:-1]
    qoff = np.cumsum(Cbq, axis=1) - Cbq       # [NB, NQ] prefix within block
    # gather chunk gcol = chunk_col0[b,q]+k  ->  dst col  dcol0[b]+qoff[b,q]+k
    g2d = np.zeros(totch, np.int64)
    for bb in range(nb):
        for q in range(nq):
            for k in range(Cbq[bb, q]):
                g2d[chunk_col0[bb, q] + k] = dcol0[bb] + qoff[bb, q] + k
    dloc_all = np.full((ncores, totslot), -1.0, np.float16)
    dslot = g2d[slot // pb] * pb + (slot % pb)
    dflat = c_e[order] * totslot + dslot
    dloc_all.reshape(-1)[dflat] = dloc_e[order].astype(np.float16)

    # dma_gather index layout: flat slot i -> [i%16, i//16], replicated later
    idx16 = idx_all.reshape(ncores, -1, 16).transpose(0, 2, 1).copy()
    # dst16 [ncores, 128(p), TOTCH]  (block-major chunk columns)
    dst16 = dloc_all.reshape(ncores, totch, pb).transpose(0, 2, 1).copy()

    # x shard rows (partition-major p*NB+b) and deg tile via the slot map
    x_pad = np.zeros((npad, D), np.float32)
    x_pad[:n] = np.asarray(x, np.float32)
    nodes_pm = node_of_slot.reshape(ncores, nb, pb).transpose(0, 2, 1)  # [c,p,b]
    xs = x_pad[nodes_pm.reshape(ncores, sh)]          # row p*NB+b = node(c,b,p)
    deg_sh = deg[nodes_pm].astype(np.float32).copy()  # [c, 128, NB]
    dinv0 = (1.0 / np.sqrt(deg)).astype(np.float32)
    z0sh = (x_pad * dinv0[:, None]).astype(np.float16)[
        nodes_pm.reshape(ncores, sh)]                 # [c, sh, D] fp16

    # host-computed z0 table in the permuted row layout (kills AllGather-0):
    # table row row_of[n] = dinv*x of node n, cols 64:128 zero
    dinv = (1.0 / np.sqrt(deg)).astype(np.float32)
    z0tab = np.zeros((npad, pb), np.float16)
    z0tab[row_of, :D] = (x_pad * dinv[:, None]).astype(np.float16)

    layout = dict(Cbq=Cbq, chunk_col0=chunk_col0, calls=calls, groups=groups,
                  group_col0=group_col0, totch=totch, totslot=totslot, cfg=cfg,
                  dcol0=dcol0, qoff=qoff, node_of_slot=node_of_slot,
                  stripe_q=bool(cfg.get("STRIPE_Q")))
    percore = dict(idx16=idx16, dst16=dst16, xs=xs, deg_sh=deg_sh,
                   z0tab=z0tab, z0sh=z0sh)
    return percore, layout


def make_in_maps(percore, W, b, cfg):
    pb, ncores = cfg["PB"], cfg["NCORES"]
    iota16 = np.broadcast_to(np.arange(pb, dtype=np.float16), (pb, pb)).copy()
    ident16 = np.eye(pb, dtype=np.float16)
    W16 = np.asarray(W, np.float16)
    b_bc = np.broadcast_to(np.asarray(b, np.float32), (pb, D)).copy()
    return [dict(x_sh=percore["xs"][c], z0sh=percore["z0sh"][c],
                 deg_sh=percore["deg_sh"][c],
                 idx16=percore["idx16"][c], dst16=percore["dst16"][c],
                 iota16=iota16, ident16=ident16, W16=W16, b_bc=b_bc,
                 z0tab=percore["z0tab"])
            for c in range(ncores)]


# ------------------------------------------------------------- bass program
def build_program(layout, no_cc=False, ablate=(), replay=1, nqueues=1,
                  single_packet=False, fuse=8, tail="act", z0direct=False,
                  mbufs=6, dbufs=6, memset=False, wbufs=2):
    """no_cc=True replaces AllGathers with local own-slice copies so the
    single-core TimelineSim (which cannot model collectives) can run.
    ablate: subset of {"gather","pe","dve","all"} — drop those stages
    (timing experiments only; results become garbage).
    replay: repeat the compute body N times inside one NEFF — timing
    probe that cancels per-dispatch overhead ((T(R)-T(1))/(R-1))."""
    bacc, tile, mybir, bass, _ = _bass_modules()
    no_gather = "gather" in ablate or "all" in ablate
    no_pe = "pe" in ablate or "all" in ablate
    no_dve = "dve" in ablate or "all" in ablate
    no_vec = "vec" in ablate or "all" in ablate
    dt = mybir.dt
    Alu = mybir.AluOpType

    cfg = layout["cfg"]
    ncores, pb = cfg["NCORES"], cfg["PB"]
    sh, nb, npad, nq, qrows = (cfg["SH"], cfg["NB"], cfg["NPAD"], cfg["NQ"],
                               cfg["QROWS"])
    Cbq = layout["Cbq"]
    chunk_col0 = layout["chunk_col0"]
    calls = layout["calls"]
    groups = layout["groups"]
    group_col0 = layout["group_col0"]
    totch = layout["totch"]
    dcol0 = layout["dcol0"]
    qoff = layout["qoff"]
    S16 = layout["totslot"] // 16
    FUSE = fuse

    nc = bacc.Bacc("TRN2", target_bir_lowering=False, debug=False,
                   num_devices=ncores, num_swdge_queues=nqueues)

    x_in = nc.dram_tensor("x_sh", [sh, D], dt.float32, kind="ExternalInput").ap()
    z0sh_in = nc.dram_tensor("z0sh", [sh, D], dt.float16,
                             kind="ExternalInput").ap()
    z0tab_in = nc.dram_tensor("z0tab", [npad, pb], dt.float16,
                              kind="ExternalInput").ap()
    deg_in = nc.dram_tensor("deg_sh", [pb, nb], dt.float32, kind="ExternalInput").ap()
    idx_in = nc.dram_tensor("idx16", [16, S16], dt.int16, kind="ExternalInput").ap()
    dst_in = nc.dram_tensor("dst16", [pb, totch], dt.float16, kind="ExternalInput").ap()
    iota_in = nc.dram_tensor("iota16", [pb, pb], dt.float16, kind="ExternalInput").ap()
    ident_in = nc.dram_tensor("ident16", [pb, pb], dt.float16,
                              kind="ExternalInput").ap()
    w_in = nc.dram_tensor("W16", [D, D], dt.float16, kind="ExternalInput").ap()
    bias_in = nc.dram_tensor("b_bc", [pb, D], dt.float32, kind="ExternalInput").ap()
    out_ext = nc.dram_tensor("out_sh", [sh, D], dt.float32, kind="ExternalOutput").ap()

    omd = float(1.0 - DELTA)
    dconst = float(DELTA)
    rg = [list(range(ncores))]

    with tile.TileContext(nc) as tc:
        with (
            tc.tile_pool(name="res", bufs=1) as res,
            tc.tile_pool(name="work", bufs=2) as work,
            tc.tile_pool(name="gtpool", bufs=wbufs) as gtpool,
            tc.tile_pool(name="mpool", bufs=mbufs) as mpool,
            tc.tile_pool(name="dpool", bufs=dbufs) as dpool,
            tc.tile_pool(name="pp", bufs=8, space="PSUM") as ps,
            tc.tile_pool(name="dram", bufs=1, space="DRAM") as dr,
        ):
            # DRAM internals (z0 table is a host-staged input; only the
            # between-steps z1 AllGather remains).  Shared buffers are
            # single-writer — allocate one per replay rep.  With stripe
            # quarters the AllGather is chunked: one collective per
            # quarter, so step-2 gathers start as soon as their quarter
            # lands.
            stripe = layout.get("stripe_q")
            st = pb // nq
            if stripe:
                z_shard1_r = [[dr.tile([sh // nq, pb], dt.float16,
                                       tag=f"zs1_{r}_{i}", name=f"zs1_{r}_{i}")
                               for i in range(nq)] for r in range(replay)]
                z_full1_r = [[dr.tile([qrows, pb], dt.float16,
                                      tag=f"zf1_{r}_{i}", name=f"zf1_{r}_{i}",
                                      addr_space="Shared") for i in range(nq)]
                             for r in range(replay)]
            else:
                z_shard1_r = [dr.tile([sh, pb], dt.float16, tag=f"zs1_{r}",
                                      name=f"zs1_{r}") for r in range(replay)]
                z_full1_r = [dr.tile([npad, pb], dt.float16, tag=f"zf1_{r}",
                                     name=f"zf1_{r}", addr_space="Shared")
                             for r in range(replay)]
            u_dram = dr.tile([sh, pb], dt.float16, tag="ud")

            # constants (iota replicated FUSE times along free dim)
            iota_f = res.tile([pb, FUSE * pb], dt.float16, tag="iota")
            for r in range(FUSE):
                nc.sync.dma_start(iota_f[:, r * pb:(r + 1) * pb], iota_in[:])
            w_t = res.tile([D, D], dt.float16, tag="w")
            nc.sync.dma_start(w_t[:], w_in[:])
            bias_t = res.tile([pb, D], dt.float32, tag="bias")
            nc.sync.dma_start(bias_t[:], bias_in[:])

            # per-node scale vectors  [128, NB]
            deg_t = res.tile([pb, nb], dt.float32, tag="deg")
            nc.sync.dma_start(deg_t[:], deg_in[:])
            rec_t = res.tile([pb, nb], dt.float32, tag="rec")
            nc.vector.reciprocal(rec_t[:], deg_t[:])
            d2d_t = res.tile([pb, nb], dt.float32, tag="d2d")
            nc.vector.tensor_scalar_mul(d2d_t[:], rec_t[:], dconst)
            c0_t = res.tile([pb, nb], dt.float32, tag="c0")
            nc.vector.tensor_scalar_add(c0_t[:], d2d_t[:], omd)
            s_t = res.tile([pb, nb], dt.float32, tag="s")
            nc.scalar.sqrt(s_t[:], deg_t[:])
            rs_t = res.tile([pb, nb], dt.float32, tag="rs")
            nc.scalar.sqrt(rs_t[:], rec_t[:])
            dinvd_t = res.tile([pb, nb], dt.float32, tag="dinvd")
            nc.vector.tensor_scalar_mul(dinvd_t[:], rs_t[:], dconst)
            c1_t = res.tile([pb, nb], dt.float32, tag="c1")
            nc.vector.scalar_tensor_tensor(
                out=c1_t[:], in0=s_t[:], scalar=omd, in1=dinvd_t[:],
                op0=Alu.mult, op1=Alu.add)
            # r = sc_in / sc_sum for the in-PSUM diag matmul fold
            ident_t = res.tile([pb, pb], dt.float16, tag="ident")
            nc.sync.dma_start(ident_t[:], ident_in[:])
            ri0_t = res.tile([pb, nb], dt.float32, tag="ri0")
            nc.vector.reciprocal(ri0_t[:], d2d_t[:])
            r0_t = res.tile([pb, nb], dt.float32, tag="r0")
            nc.vector.tensor_tensor(r0_t[:], c0_t[:], ri0_t[:], op=Alu.mult)
            ri1_t = res.tile([pb, nb], dt.float32, tag="ri1")
            nc.vector.reciprocal(ri1_t[:], dinvd_t[:])
            r1_t = res.tile([pb, nb], dt.float32, tag="r1")
            nc.vector.tensor_tensor(r1_t[:], c1_t[:], ri1_t[:], op=Alu.mult)

            # gather indices (replicate 16 -> 128 partitions) + dst locals
            idx_t = res.tile([pb, S16], dt.int16, tag="idx")
            for r in range(8):
                nc.sync.dma_start(idx_t[16 * r:16 * (r + 1), :], idx_in[:])
            dst_t = res.tile([pb, totch], dt.float16, tag="dst")
            nc.sync.dma_start(dst_t[:], dst_in[:])

            # z0 = dinv * x   (body below replayed `replay` times — the
            # computation is idempotent, so results stay correct)
            for _rep in range(replay):
              z_res = [res.tile([pb, nb * pb], dt.float16, tag=f"zr{i}", name=f"zr{i}")
                       for i in range(2)]
              if memset:
                  nc.gpsimd.memset(z_res[0][:], 0.0)
                  nc.gpsimd.memset(z_res[1][:], 0.0)
              if z0direct:
                  nc.sync.dma_start(
                      z_res[0][:].rearrange("p (b j) -> p b j", j=pb)[:, :, 0:D],
                      z0sh_in.rearrange("(p b) j -> p b j", p=pb))
              else:
                  x_res = res.tile([pb, nb * D], dt.float32, tag="xu")
                  nc.sync.dma_start(
                      x_res[:].rearrange("p (b j) -> p b j", j=D),
                      x_in.rearrange("(p b) j -> p b j", p=pb))
                  for b in range(nb):
                      nc.scalar.activation(
                          out=z_res[0][:, b * pb:b * pb + D],
                          in_=x_res[:, b * D:(b + 1) * D],
                          func=mybir.ActivationFunctionType.Copy,
                          scale=rs_t[:, b:b + 1])

              u_res = res.tile([pb, nb * pb], dt.float16, tag="xu")
              if memset:
                  nc.gpsimd.memset(u_res[:], 0.0)
              z_shard1, z_full1 = z_shard1_r[_rep], z_full1_r[_rep]

              # two propagation steps
              for it in range(2):
                  if it == 0:
                      win = lambda q: z0tab_in[q * qrows:(q + 1) * qrows, :]
                  elif stripe:
                      win = lambda q: z_full1[q][:]
                  else:
                      win = lambda q: z_full1[q * qrows:(q + 1) * qrows, :]
                  sc_sum = d2d_t if it == 0 else dinvd_t
                  r_t = r0_t if it == 0 else r1_t
                  src_res = z_res[it]
                  dst_res = z_res[1] if it == 0 else u_res
                  for gi, blocks in enumerate(groups):
                      g0 = group_col0[gi]
                      gw = group_col0[gi + 1] - g0
                      gt = gtpool.tile([pb, gw * pb], dt.float16, tag="gt")
                      if no_gather:
                          nc.vector.memset(gt[:, 0:64], 0.0)
                      for (gg, q, col0, ncols) in calls:
                          if gg != gi or no_gather:
                              continue
                          nidx = ncols * pb
                          nc.gpsimd.dma_gather(
                              gt[:, (col0 - g0) * pb:(col0 - g0 + ncols) * pb]
                              .rearrange("p (c e) -> p c e", e=pb),
                              win(q),
                              idx_t[:, col0 * 8:(col0 + ncols) * 8],
                              nidx, nidx, pb, single_packet=single_packet,
                              queue_num=q % nqueues)
                      for b in blocks:
                          # (gather col, dst col) per chunk, dst cols consecutive
                          chunks = []
                          for q in range(nq):
                              for k in range(Cbq[b, q]):
                                  chunks.append((chunk_col0[b, q] + k,
                                                 dcol0[b] + qoff[b, q] + k))
                          psum_t = ps.tile([pb, D], dt.float32, tag="ps")
                          if no_pe:
                              nc.vector.memset(psum_t[:], 0.0)
                          nch = len(chunks)
                          # diag(r) term: psum += diag(sc_in/sc_sum) @ src
                          dg_t = dpool.tile([pb, pb], dt.float16, tag="dg")
                          if not no_dve:
                              nc.vector.tensor_scalar(
                                  out=dg_t[:], in0=ident_t[:],
                                  scalar1=r_t[:, b:b + 1], scalar2=None,
                                  op0=Alu.mult)
                          # fused one-hot build, FUSE chunks per DVE instruction
                          for f0 in range(0, nch, FUSE):
                              f1 = min(f0 + FUSE, nch)
                              nf = f1 - f0
                              dc = chunks[f0][1]
                              m_t = mpool.tile([pb, FUSE * pb], dt.float16, tag="m")
                              if not no_dve:
                                  nc.vector.tensor_tensor(
                                      out=m_t[:, :nf * pb].rearrange(
                                          "p (c e) -> p c e", e=pb),
                                      in0=iota_f[:, :nf * pb].rearrange(
                                          "p (c e) -> p c e", e=pb),
                                      in1=dst_t[:, dc:dc + nf].to_broadcast(
                                          [pb, nf, pb]),
                                      op=Alu.is_equal)
                              if no_pe:
                                  continue
                              for j in range(nf):
                                  ci = f0 + j
                                  gcol = chunks[ci][0]
                                  nc.tensor.matmul(
                                      out=psum_t[:], lhsT=m_t[:, j * pb:(j + 1) * pb],
                                      rhs=gt[:, (gcol - g0) * pb:(gcol - g0) * pb + D],
                                      start=(ci == 0), stop=False)
                          if not no_pe:
                              nc.tensor.matmul(
                                  out=psum_t[:], lhsT=dg_t[:],
                                  rhs=src_res[:, b * pb:b * pb + D],
                                  start=False, stop=True)
                          if no_vec:
                              continue
                          # dst = sc_sum * psum  (ACT: PSUM read + per-
                          # partition scale, keeps DVE free of PSUM waits)
                          if tail == "act":
                              nc.scalar.activation(
                                  out=dst_res[:, b * pb:b * pb + D],
                                  in_=psum_t[:],
                                  func=mybir.ActivationFunctionType.Copy,
                                  scale=sc_sum[:, b:b + 1])
                          else:
                              nc.vector.tensor_scalar(
                                  out=dst_res[:, b * pb:b * pb + D],
                                  in0=psum_t[:], scalar1=sc_sum[:, b:b + 1],
                                  scalar2=None, op0=Alu.mult)
                  if it == 0:
                      if stripe:
                          for i in range(nq):
                              zp = z_shard1[i]
                              nc.sync.dma_start(
                                  zp[:].rearrange("(p b) j -> p b j", p=st),
                                  z_res[1][st * i:st * (i + 1), :]
                                  .rearrange("p (b j) -> p b j", j=pb))
                              if no_cc:
                                  nc.gpsimd.dma_start(
                                      z_full1[i][0:sh // nq, :], zp[:])
                              else:
                                  nc.gpsimd.collective_compute(
                                      "AllGather", Alu.bypass,
                                      replica_groups=rg,
                                      ins=[zp[:]], outs=[z_full1[i][:]])
                      else:
                          nc.sync.dma_start(
                              z_shard1[:].rearrange("(p b) j -> p b j", p=pb),
                              z_res[1][:].rearrange("p (b j) -> p b j", j=pb))
                          if no_cc:
                              nc.gpsimd.dma_start(z_full1[0:sh, :], z_shard1[:])
                          else:
                              nc.gpsimd.collective_compute(
                                  "AllGather", Alu.bypass, replica_groups=rg,
                                  ins=[z_shard1[:]], outs=[z_full1[:]])
                  else:
                      nc.sync.dma_start(
                          u_dram[:].rearrange("(p b) j -> p b j", p=pb),
                          u_res[:].rearrange("p (b j) -> p b j", j=pb))

              # out = u @ W + b   (transposed reload of u gives lhsT)
              ut = res.tile([pb, sh], dt.float16, tag="zr0")
              nc.sync.dma_start(out=ut[:], in_=u_dram[:], transpose=True)
              out_res = work.tile([pb, nb * D], dt.float32, tag="gt")
              for i in range(nb):
                  psj = ps.tile([pb, D], dt.float32, tag="ps")
                  nc.tensor.matmul(out=psj[:], lhsT=ut[0:D, i * pb:(i + 1) * pb],
                                   rhs=w_t[:], start=True, stop=True)
                  nc.vector.tensor_tensor(
                      out=out_res[:, i * D:(i + 1) * D], in0=psj[:],
                      in1=bias_t[:], op=Alu.add)
              nc.sync.dma_start(
                  out_ext.rearrange("(p i) j -> p i j", p=pb),
                  out_res[:].rearrange("p (i j) -> p i j", j=D))

    nc.compile()
    return nc


# ---------------------------------------------------------------- unpermute
def unpermute_out(results, cfg, node_of_slot=None):
    ncores, pb, sh, nb, npad = (cfg["NCORES"], cfg["PB"], cfg["SH"], cfg["NB"],
                                cfg["NPAD"])
    if node_of_slot is None:
        node_of_slot = np.arange(npad, dtype=np.int64)
    out = np.empty((npad, D), np.float32)
    rp = np.arange(sh)
    p = rp // nb            # out_sh row r' = p*NB + i ...
    i = rp % nb
    r = i * pb + p          # ... holds u-flat row r = i*128 + p
    p2 = r // nb            # u-flat row r = p2*NB + b2  (partition major)
    b2 = r % nb
    slot_loc = b2 * pb + p2
    for c in range(ncores):
        out[node_of_slot[c * sh + slot_loc]] = results[c]["out_sh"][rp]
    return out[:cfg["N"]]


# ------------------------------------------------------------------- entry
def kernel(**inputs):
    x = np.asarray(inputs["x"])
    edge_index = np.asarray(inputs["edge_index"])
    W = np.asarray(inputs["W"])
    b = np.asarray(inputs["b"])

    _, _, _, _, run_bass_kernel_spmd = _bass_modules()
    cfg = _finish_cfg(DEFAULT_CFG)

    percore, layout = host_prep(x, edge_index, cfg)
    in_maps = make_in_maps(percore, W, b, cfg)
    key = layout["Cbq"].tobytes()
    if key not in _CACHE:
        _CACHE[key] = build_program(layout, nqueues=4)
    nc = _CACHE[key]

    res = run_bass_kernel_spmd(nc, in_maps, core_ids=list(range(cfg["NCORES"])))
    return unpermute_out(res.results, cfg, layout["node_of_slot"])



# revision 37
# speedup vs baseline: 1.4480x; 1.4480x over previous
"""DGConv (GCN diffusion, K=2 steps) on 8 Trainium2 NeuronCores.

Math (equivalent reformulation of the reference):
  deg_i  = indegree_i + 1 (self loop),  dinv = deg^-1/2
  z      = dinv * x            (per-edge weight dinv_s*dinv_d folds away)
  S(z)_i = sum_{(j->i) in E} z_j         (unweighted segment sum, no self loops)
  step:  z' = (1-d)*z + d*(1/deg)*(S(z) + z),   d = T/K
  out    = (sqrt(deg) * z_K) @ W + b
         = [S2*(d*dinv) + z1*((1-d)sqrt(deg)+d*dinv)] @ W + b
           (the Linear commutes with all row-wise scaling and with S)

Distribution: dst-node ranges sharded 8 ways (SH=12544 nodes/core).  Each
core gathers z[src] for its edges from a replicated HBM table via dma_gather
(int16 indices -> 4 quarter windows of the table), segment-sums with one-hot
matmuls on the PE (128-dst-node blocks, PSUM accumulation), and AllGathers
the updated z shard between the two propagation steps.

Table rows are permuted partition-major (node (c,b,p) -> row c*SH+p*NB+b) so
internal DMAs are contiguous per partition; the host un-permutes at the end.
Table rows are 128 fp16 (=256B, dma_gather granularity); cols 64:128 unused.
"""

import numpy as np

# ---------------------------------------------------------------- constants
N = 100000
D = 64
T = 5.27
KSTEPS = 2
DELTA = T / KSTEPS

DEFAULT_CFG = dict(
    N=100000,
    NCORES=8,
    PB=128,        # dst nodes per block (one-hot width)
    NB=100,        # blocks per core (1.7% slack under the 4-chunk cap)
    GB=2,          # blocks per gather group
    NQ=4,          # int16 index windows
    BALANCE=True,  # greedy per-(block,quarter) edge-count balancing
    STRIPE_Q=True, # lane-stripe quarters -> chunked (per-quarter) AllGather
)

_CACHE = {}


def _finish_cfg(cfg):
    cfg = dict(cfg)
    cfg["SH"] = cfg["PB"] * cfg["NB"]
    cfg["NPAD"] = cfg["SH"] * cfg["NCORES"]
    assert cfg["NPAD"] % cfg["NQ"] == 0
    cfg["QROWS"] = cfg["NPAD"] // cfg["NQ"]
    assert cfg["QROWS"] <= 32768
    return cfg


def _bass_modules():
    import sys
    if "/opt/trn_rl_repo" not in sys.path:
        sys.path.insert(0, "/opt/trn_rl_repo")
    import concourse.bacc as bacc
    import concourse.tile as tile
    import concourse.mybir as mybir
    from concourse import bass
    from concourse.bass_utils import run_bass_kernel_spmd
    return bacc, tile, mybir, bass, run_bass_kernel_spmd


# ---------------------------------------------------------------- host prep
def _core_of_nodes(cfg):
    """Node -> core. Real nodes split evenly (N/NCORES each); pad nodes
    (N..NPAD) distributed evenly so no core carries extra real load."""
    n, ncores, npad = cfg["N"], cfg["NCORES"], cfg["NPAD"]
    real = n // ncores
    pads = (npad - n) // ncores
    core_of = np.empty(npad, np.int64)
    core_of[:n] = np.minimum(np.arange(n) // real, ncores - 1)
    core_of[n:] = np.arange(npad - n) // pads
    return core_of


def _balanced_assignment(dst, src, core_of, cfg):
    """Per core, pack its SH nodes into NB blocks of PB nodes, balancing the
    per-(block, src-quarter) edge counts (greedy vector LPT with soft cap).
    A node's gather quarter depends only on its core (core table rows share
    qrows windows), so packing cannot disturb quarter labels.
    Returns node_of_slot [NPAD]: slot c*SH + b*PB + p -> node id."""
    ncores, pb, sh, nb = cfg["NCORES"], cfg["PB"], cfg["SH"], cfg["NB"]
    npad, nq, qrows = cfg["NPAD"], cfg["NQ"], cfg["QROWS"]
    q_src = core_of[src] // (qrows // sh)      # quarter = f(core of src)
    dq = np.bincount(dst * nq + q_src, minlength=npad * nq).reshape(npad, nq)
    # global target chunks per (block, quarter); placements below target*PB
    # are free (the cross-core max will be at target anyway)
    tgt = max(1, -(-dq.sum() // (ncores * nb * nq * pb)))
    cap = tgt * pb
    node_of_slot = np.empty(npad, np.int64)
    BIG = 1 << 22
    bidx = np.arange(nb)
    for c in range(ncores):
        cnodes = np.where(core_of == c)[0]
        assert cnodes.shape[0] == sh
        v = dq[cnodes].astype(np.int64)                   # [SH, NQ]
        order = np.argsort(-v.sum(axis=1), kind="stable")
        sums = np.zeros((nb, nq), np.int64)
        cnts = np.zeros(nb, np.int64)
        binL = np.empty(sh, np.int64)                      # node -> block
        for nloc in order:
            vi = v[nloc]
            cand = sums + vi                               # [NB, NQ]
            # cost = chunk-count increase ABOVE the global target
            dch = (np.maximum((cand + pb - 1) // pb, tgt)
                   - np.maximum((sums + pb - 1) // pb, tgt)).sum(axis=1)
            # steer overflow to the same (low) block index on every core
            over = (cand > cap).any(axis=1)
            score = dch * BIG + over * bidx * 2048 + cand.max(axis=1)
            score[cnts >= pb] = 1 << 60
            bsel = int(np.argmin(score))
            sums[bsel] += vi
            binL[nloc] = bsel
            cnts[bsel] += 1
        lane = np.zeros(nb, np.int64)
        slots = np.empty(sh, np.int64)
        for nloc in range(sh):
            b = binL[nloc]
            slots[nloc] = b * pb + lane[b]
            lane[b] += 1
        node_of_slot[c * sh + slots] = cnodes
    return node_of_slot


def _stripe_lanes(dst, src, core_of, node_of_slot, cfg):
    """Reassign lanes within each (core, block) so per-(dst-core, dst-block,
    stripe) edge counts stay under the global chunk cap.  A node's lane
    stripe (lane//32) is the gather quarter of all its OUT-edges; greedy
    over nodes by out-degree, 32-per-stripe quota per block."""
    ncores, pb, sh, nb = cfg["NCORES"], cfg["PB"], cfg["SH"], cfg["NB"]
    nq, npad = cfg["NQ"], cfg["NPAD"]
    st = pb // nq
    E = dst.shape[0]
    slot_of_node = np.empty(npad, np.int64)
    slot_of_node[node_of_slot] = np.arange(npad)
    b_of = (slot_of_node % sh) // pb
    cell_e = core_of[dst] * nb + b_of[dst]
    eorder = np.argsort(src, kind="stable")
    src_s = src[eorder]
    cell_s = cell_e[eorder]
    starts = np.searchsorted(src_s, np.arange(npad + 1))
    outdeg = np.diff(starts)
    tgt = max(1, -(-E // (ncores * nb * nq * pb)))
    cap = tgt * pb
    cnt = np.zeros((ncores * nb, nq), np.int32)
    quota = np.full((ncores * nb, nq), st, np.int32)
    lane_q = np.empty(npad, np.int8)
    BIG = np.float64(1e18)
    for n in np.argsort(-outdeg, kind="stable"):
        qb = core_of[n] * nb + b_of[n]
        s0, s1 = starts[n], starts[n + 1]
        if s1 > s0:
            u, m = np.unique(cell_s[s0:s1], return_counts=True)
            base = cnt[u]                                    # [k, nq]
            cand = base + m[:, None]
            over = (np.maximum(cand - cap, 0)
                    - np.maximum(base - cap, 0)).sum(axis=0)
            score = over * 1e6 + cand.max(axis=0)
        else:
            score = np.zeros(nq)
        score = np.where(quota[qb] > 0, score, BIG)
        q = int(np.argmin(score))
        lane_q[n] = q
        quota[qb, q] -= 1
        if s1 > s0:
            np.add.at(cnt, (cell_s[s0:s1], q), 1)
    # lanes: stripe-major fill within each (core, block)
    keys = (core_of * nb + b_of) * nq + lane_q
    order2 = np.lexsort((np.arange(npad), keys))
    ks = keys[order2]
    first = np.ones(npad, bool)
    first[1:] = ks[1:] != ks[:-1]
    gs = np.maximum.accumulate(np.where(first, np.arange(npad), 0))
    rank = np.arange(npad) - gs
    c2, b2 = core_of[order2], b_of[order2]
    lane2 = lane_q[order2] * st + rank
    out = np.empty(npad, np.int64)
    out[c2 * sh + b2 * pb + lane2] = order2
    return out


def host_prep(x, edge_index, cfg):
    """Per-core input arrays + static chunk layout (shared across cores)."""
    n, ncores, pb = cfg["N"], cfg["NCORES"], cfg["PB"]
    sh, nb, npad = cfg["SH"], cfg["NB"], cfg["NPAD"]
    nq, qrows, gb = cfg["NQ"], cfg["QROWS"], cfg["GB"]

    src = np.asarray(edge_index[0], np.int64)
    dst = np.asarray(edge_index[1], np.int64)
    E = src.shape[0]

    deg = 1.0 + np.bincount(dst, minlength=npad)[:npad].astype(np.float32)

    if cfg.get("BALANCE"):
        core_of = _core_of_nodes(cfg)
        node_of_slot = _balanced_assignment(dst, src, core_of, cfg)
    else:
        core_of = np.arange(npad, dtype=np.int64) // sh
        node_of_slot = np.arange(npad, dtype=np.int64)
    if cfg.get("STRIPE_Q"):
        node_of_slot = _stripe_lanes(dst, src, core_of, node_of_slot, cfg)
    # inverse: node -> (block, lane) within its core
    slot_of_node = np.empty(npad, np.int64)
    slot_of_node[node_of_slot] = np.arange(npad)
    b_of = (slot_of_node % sh) // pb
    p_of = (slot_of_node % sh) % pb

    c_e = core_of[dst]
    b_e = b_of[dst]
    dloc_e = p_of[dst].astype(np.int32)
    if cfg.get("STRIPE_Q"):
        # stripe quarters: q = lane//(PB/NQ); table row
        #   q*(NPAD/NQ) + c*(SH/NQ) + (p % (PB/NQ))*NB + b
        # so quarter q of the table is exactly what chunk-AllGather q
        # delivers (each core sends lanes [32q:32q+32) of its z shard).
        st = pb // nq
        row_of = ((p_of // st) * (npad // nq) + core_of * (sh // nq)
                  + (p_of % st) * nb + b_of)          # node id -> table row
    else:
        # permuted table row: (c2, b2, p2) -> c2*SH + p2*NB + b2
        row_of = core_of * sh + p_of * nb + b_of
    pr_src = row_of[src]
    q_e = (pr_src // qrows).astype(np.int32)
    qi_e = (pr_src % qrows).astype(np.int32)

    # chunk budget per (block, quarter): max over cores
    key = ((c_e * nb + b_e) * nq + q_e).astype(np.int64)
    cnt = np.bincount(key, minlength=ncores * nb * nq).reshape(ncores, nb, nq)
    Cbq = -(-cnt.max(axis=0) // pb)              # [NB, NQ]

    groups = [range(g0, min(g0 + gb, nb)) for g0 in range(0, nb, gb)]
    col = 0
    chunk_col0 = np.zeros((nb, nq), np.int64)
    calls = []                                   # (gi, q, col0, ncols)
    group_col0 = []
    for gi, blocks in enumerate(groups):
        group_col0.append(col)
        for q in range(nq):
            start = col
            for bb in blocks:
                chunk_col0[bb, q] = col
                col += Cbq[bb, q]
            if col > start:
                calls.append((gi, q, start, col - start))
    totch = int(col)
    totslot = totch * pb
    group_col0.append(totch)

    # slot assignment: edges sorted by (c,b,q); rank within each segment
    order = np.lexsort((q_e, b_e, c_e))
    key_s = key[order]
    first = np.ones(E, bool)
    first[1:] = key_s[1:] != key_s[:-1]
    seg_start = np.maximum.accumulate(np.where(first, np.arange(E), 0))
    rank = np.arange(E) - seg_start
    slot = chunk_col0[b_e[order], q_e[order]] * pb + rank

    idx_all = np.zeros((ncores, totslot), np.int16)
    flat = c_e[order] * totslot + slot
    idx_all.reshape(-1)[flat] = qi_e[order].astype(np.int16)

    # dst-local table in BLOCK-MAJOR chunk order (for fused one-hot builds):
    # block b's chunks occupy consecutive columns dcol0[b] + qoff[b,q] + k
    Cb = Cbq.sum(axis=1)                      # chunks per block
    dcol0 = np.concatenate([[0], np.cumsum(Cb)])[:-1]
    qoff = np.cumsum(Cbq, axis=1) - Cbq       # [NB, NQ] prefix within block
    # gather chunk gcol = chunk_col0[b,q]+k  ->  dst col  dcol0[b]+qoff[b,q]+k
    g2d = np.zeros(totch, np.int64)
    for bb in range(nb):
        for q in range(nq):
            for k in range(Cbq[bb, q]):
                g2d[chunk_col0[bb, q] + k] = dcol0[bb] + qoff[bb, q] + k
    dloc_all = np.full((ncores, totslot), -1.0, np.float16)
    dslot = g2d[slot // pb] * pb + (slot % pb)
    dflat = c_e[order] * totslot + dslot
    dloc_all.reshape(-1)[dflat] = dloc_e[order].astype(np.float16)

    # dma_gather index layout: flat slot i -> [i%16, i//16], replicated later
    idx16 = idx_all.reshape(ncores, -1, 16).transpose(0, 2, 1).copy()
    # dst16 [ncores, 128(p), TOTCH]  (block-major chunk columns)
    dst16 = dloc_all.reshape(ncores, totch, pb).transpose(0, 2, 1).copy()

    # x shard rows (partition-major p*NB+b) and deg tile via the slot map
    x_pad = np.zeros((npad, D), np.float32)
    x_pad[:n] = np.asarray(x, np.float32)
    nodes_pm = node_of_slot.reshape(ncores, nb, pb).transpose(0, 2, 1)  # [c,p,b]
    xs = x_pad[nodes_pm.reshape(ncores, sh)]          # row p*NB+b = node(c,b,p)
    deg_sh = deg[nodes_pm].astype(np.float32).copy()  # [c, 128, NB]
    dinv0 = (1.0 / np.sqrt(deg)).astype(np.float32)
    z0sh = (x_pad * dinv0[:, None]).astype(np.float16)[
        nodes_pm.reshape(ncores, sh)]                 # [c, sh, D] fp16

    # host-computed z0 table in the permuted row layout (kills AllGather-0):
    # table row row_of[n] = dinv*x of node n, cols 64:128 zero
    dinv = (1.0 / np.sqrt(deg)).astype(np.float32)
    z0tab = np.zeros((npad, pb), np.float16)
    z0tab[row_of, :D] = (x_pad * dinv[:, None]).astype(np.float16)

    layout = dict(Cbq=Cbq, chunk_col0=chunk_col0, calls=calls, groups=groups,
                  group_col0=group_col0, totch=totch, totslot=totslot, cfg=cfg,
                  dcol0=dcol0, qoff=qoff, node_of_slot=node_of_slot,
                  stripe_q=bool(cfg.get("STRIPE_Q")))
    percore = dict(idx16=idx16, dst16=dst16, xs=xs, deg_sh=deg_sh,
                   z0tab=z0tab, z0sh=z0sh)
    return percore, layout


def make_in_maps(percore, W, b, cfg):
    pb, ncores = cfg["PB"], cfg["NCORES"]
    iota16 = np.broadcast_to(np.arange(pb, dtype=np.float16), (pb, pb)).copy()
    ident16 = np.eye(pb, dtype=np.float16)
    W16 = np.asarray(W, np.float16)
    b_bc = np.broadcast_to(np.asarray(b, np.float32), (pb, D)).copy()
    return [dict(x_sh=percore["xs"][c], z0sh=percore["z0sh"][c],
                 deg_sh=percore["deg_sh"][c],
                 idx16=percore["idx16"][c], dst16=percore["dst16"][c],
                 iota16=iota16, ident16=ident16, W16=W16, b_bc=b_bc,
                 z0tab=percore["z0tab"])
            for c in range(ncores)]


# ------------------------------------------------------------- bass program
def build_program(layout, no_cc=False, ablate=(), replay=1, nqueues=1,
                  single_packet=False, fuse=8, tail="act", z0direct=False,
                  mbufs=12, dbufs=12, memset=False, wbufs=2, qrot=False):
    """no_cc=True replaces AllGathers with local own-slice copies so the
    single-core TimelineSim (which cannot model collectives) can run.
    ablate: subset of {"gather","pe","dve","all"} — drop those stages
    (timing experiments only; results become garbage).
    replay: repeat the compute body N times inside one NEFF — timing
    probe that cancels per-dispatch overhead ((T(R)-T(1))/(R-1))."""
    bacc, tile, mybir, bass, _ = _bass_modules()
    no_gather = "gather" in ablate or "all" in ablate
    no_pe = "pe" in ablate or "all" in ablate
    no_dve = "dve" in ablate or "all" in ablate
    no_vec = "vec" in ablate or "all" in ablate
    dt = mybir.dt
    Alu = mybir.AluOpType

    cfg = layout["cfg"]
    ncores, pb = cfg["NCORES"], cfg["PB"]
    sh, nb, npad, nq, qrows = (cfg["SH"], cfg["NB"], cfg["NPAD"], cfg["NQ"],
                               cfg["QROWS"])
    Cbq = layout["Cbq"]
    chunk_col0 = layout["chunk_col0"]
    calls = layout["calls"]
    groups = layout["groups"]
    group_col0 = layout["group_col0"]
    totch = layout["totch"]
    dcol0 = layout["dcol0"]
    qoff = layout["qoff"]
    S16 = layout["totslot"] // 16
    FUSE = fuse

    nc = bacc.Bacc("TRN2", target_bir_lowering=False, debug=False,
                   num_devices=ncores, num_swdge_queues=nqueues)

    x_in = nc.dram_tensor("x_sh", [sh, D], dt.float32, kind="ExternalInput").ap()
    z0sh_in = nc.dram_tensor("z0sh", [sh, D], dt.float16,
                             kind="ExternalInput").ap()
    z0tab_in = nc.dram_tensor("z0tab", [npad, pb], dt.float16,
                              kind="ExternalInput").ap()
    deg_in = nc.dram_tensor("deg_sh", [pb, nb], dt.float32, kind="ExternalInput").ap()
    idx_in = nc.dram_tensor("idx16", [16, S16], dt.int16, kind="ExternalInput").ap()
    dst_in = nc.dram_tensor("dst16", [pb, totch], dt.float16, kind="ExternalInput").ap()
    iota_in = nc.dram_tensor("iota16", [pb, pb], dt.float16, kind="ExternalInput").ap()
    ident_in = nc.dram_tensor("ident16", [pb, pb], dt.float16,
                              kind="ExternalInput").ap()
    w_in = nc.dram_tensor("W16", [D, D], dt.float16, kind="ExternalInput").ap()
    bias_in = nc.dram_tensor("b_bc", [pb, D], dt.float32, kind="ExternalInput").ap()
    out_ext = nc.dram_tensor("out_sh", [sh, D], dt.float32, kind="ExternalOutput").ap()

    omd = float(1.0 - DELTA)
    dconst = float(DELTA)
    rg = [list(range(ncores))]

    with tile.TileContext(nc) as tc:
        with (
            tc.tile_pool(name="res", bufs=1) as res,
            tc.tile_pool(name="work", bufs=2) as work,
            tc.tile_pool(name="gtpool", bufs=wbufs) as gtpool,
            tc.tile_pool(name="mpool", bufs=mbufs) as mpool,
            tc.tile_pool(name="dpool", bufs=dbufs) as dpool,
            tc.tile_pool(name="pp", bufs=8, space="PSUM") as ps,
            tc.tile_pool(name="dram", bufs=1, space="DRAM") as dr,
        ):
            # DRAM internals (z0 table is a host-staged input; only the
            # between-steps z1 AllGather remains).  Shared buffers are
            # single-writer — allocate one per replay rep.  With stripe
            # quarters the AllGather is chunked: one collective per
            # quarter, so step-2 gathers start as soon as their quarter
            # lands.
            stripe = layout.get("stripe_q")
            st = pb // nq
            if stripe:
                z_shard1_r = [[dr.tile([sh // nq, pb], dt.float16,
                                       tag=f"zs1_{r}_{i}", name=f"zs1_{r}_{i}")
                               for i in range(nq)] for r in range(replay)]
                z_full1_r = [[dr.tile([qrows, pb], dt.float16,
                                      tag=f"zf1_{r}_{i}", name=f"zf1_{r}_{i}",
                                      addr_space="Shared") for i in range(nq)]
                             for r in range(replay)]
            else:
                z_shard1_r = [dr.tile([sh, pb], dt.float16, tag=f"zs1_{r}",
                                      name=f"zs1_{r}") for r in range(replay)]
                z_full1_r = [dr.tile([npad, pb], dt.float16, tag=f"zf1_{r}",
                                     name=f"zf1_{r}", addr_space="Shared")
                             for r in range(replay)]
            u_dram = dr.tile([sh, pb], dt.float16, tag="ud")

            # constants (iota replicated FUSE times along free dim)
            iota_f = res.tile([pb, FUSE * pb], dt.float16, tag="iota")
            for r in range(FUSE):
                nc.sync.dma_start(iota_f[:, r * pb:(r + 1) * pb], iota_in[:])
            w_t = res.tile([D, D], dt.float16, tag="w")
            nc.sync.dma_start(w_t[:], w_in[:])
            bias_t = res.tile([pb, D], dt.float32, tag="bias")
            nc.sync.dma_start(bias_t[:], bias_in[:])

            # per-node scale vectors  [128, NB]
            deg_t = res.tile([pb, nb], dt.float32, tag="deg")
            nc.sync.dma_start(deg_t[:], deg_in[:])
            rec_t = res.tile([pb, nb], dt.float32, tag="rec")
            nc.vector.reciprocal(rec_t[:], deg_t[:])
            d2d_t = res.tile([pb, nb], dt.float32, tag="d2d")
            nc.vector.tensor_scalar_mul(d2d_t[:], rec_t[:], dconst)
            c0_t = res.tile([pb, nb], dt.float32, tag="c0")
            nc.vector.tensor_scalar_add(c0_t[:], d2d_t[:], omd)
            s_t = res.tile([pb, nb], dt.float32, tag="s")
            nc.scalar.sqrt(s_t[:], deg_t[:])
            rs_t = res.tile([pb, nb], dt.float32, tag="rs")
            nc.scalar.sqrt(rs_t[:], rec_t[:])
            dinvd_t = res.tile([pb, nb], dt.float32, tag="dinvd")
            nc.vector.tensor_scalar_mul(dinvd_t[:], rs_t[:], dconst)
            c1_t = res.tile([pb, nb], dt.float32, tag="c1")
            nc.vector.scalar_tensor_tensor(
                out=c1_t[:], in0=s_t[:], scalar=omd, in1=dinvd_t[:],
                op0=Alu.mult, op1=Alu.add)
            # r = sc_in / sc_sum for the in-PSUM diag matmul fold
            ident_t = res.tile([pb, pb], dt.float16, tag="ident")
            nc.sync.dma_start(ident_t[:], ident_in[:])
            ri0_t = res.tile([pb, nb], dt.float32, tag="ri0")
            nc.vector.reciprocal(ri0_t[:], d2d_t[:])
            r0_t = res.tile([pb, nb], dt.float32, tag="r0")
            nc.vector.tensor_tensor(r0_t[:], c0_t[:], ri0_t[:], op=Alu.mult)
            ri1_t = res.tile([pb, nb], dt.float32, tag="ri1")
            nc.vector.reciprocal(ri1_t[:], dinvd_t[:])
            r1_t = res.tile([pb, nb], dt.float32, tag="r1")
            nc.vector.tensor_tensor(r1_t[:], c1_t[:], ri1_t[:], op=Alu.mult)

            # gather indices (replicate 16 -> 128 partitions) + dst locals
            idx_t = res.tile([pb, S16], dt.int16, tag="idx")
            for r in range(8):
                nc.sync.dma_start(idx_t[16 * r:16 * (r + 1), :], idx_in[:])
            dst_t = res.tile([pb, totch], dt.float16, tag="dst")
            nc.sync.dma_start(dst_t[:], dst_in[:])

            # z0 = dinv * x   (body below replayed `replay` times — the
            # computation is idempotent, so results stay correct)
            for _rep in range(replay):
              z_res = [res.tile([pb, nb * pb], dt.float16, tag=f"zr{i}", name=f"zr{i}")
                       for i in range(2)]
              if memset:
                  nc.gpsimd.memset(z_res[0][:], 0.0)
                  nc.gpsimd.memset(z_res[1][:], 0.0)
              if z0direct:
                  nc.sync.dma_start(
                      z_res[0][:].rearrange("p (b j) -> p b j", j=pb)[:, :, 0:D],
                      z0sh_in.rearrange("(p b) j -> p b j", p=pb))
              else:
                  x_res = res.tile([pb, nb * D], dt.float32, tag="xu")
                  nc.sync.dma_start(
                      x_res[:].rearrange("p (b j) -> p b j", j=D),
                      x_in.rearrange("(p b) j -> p b j", p=pb))
                  for b in range(nb):
                      nc.scalar.activation(
                          out=z_res[0][:, b * pb:b * pb + D],
                          in_=x_res[:, b * D:(b + 1) * D],
                          func=mybir.ActivationFunctionType.Copy,
                          scale=rs_t[:, b:b + 1])

              u_res = res.tile([pb, nb * pb], dt.float16, tag="xu")
              if memset:
                  nc.gpsimd.memset(u_res[:], 0.0)
              z_shard1, z_full1 = z_shard1_r[_rep], z_full1_r[_rep]

              # two propagation steps
              for it in range(2):
                  if it == 0:
                      win = lambda q: z0tab_in[q * qrows:(q + 1) * qrows, :]
                  elif stripe:
                      win = lambda q: z_full1[q][:]
                  else:
                      win = lambda q: z_full1[q * qrows:(q + 1) * qrows, :]
                  sc_sum = d2d_t if it == 0 else dinvd_t
                  r_t = r0_t if it == 0 else r1_t
                  src_res = z_res[it]
                  dst_res = z_res[1] if it == 0 else u_res
                  for gi, blocks in enumerate(groups):
                      g0 = group_col0[gi]
                      gw = group_col0[gi + 1] - g0
                      gt = gtpool.tile([pb, gw * pb], dt.float16, tag="gt")
                      if no_gather:
                          nc.vector.memset(gt[:, 0:64], 0.0)
                      for (gg, q, col0, ncols) in calls:
                          if gg != gi or no_gather:
                              continue
                          nidx = ncols * pb
                          nc.gpsimd.dma_gather(
                              gt[:, (col0 - g0) * pb:(col0 - g0 + ncols) * pb]
                              .rearrange("p (c e) -> p c e", e=pb),
                              win(q),
                              idx_t[:, col0 * 8:(col0 + ncols) * 8],
                              nidx, nidx, pb, single_packet=single_packet,
                              queue_num=(q + gi if qrot else q) % nqueues)
                      for b in blocks:
                          # (gather col, dst col) per chunk, dst cols consecutive
                          chunks = []
                          for q in range(nq):
                              for k in range(Cbq[b, q]):
                                  chunks.append((chunk_col0[b, q] + k,
                                                 dcol0[b] + qoff[b, q] + k))
                          psum_t = ps.tile([pb, D], dt.float32, tag="ps")
                          if no_pe:
                              nc.vector.memset(psum_t[:], 0.0)
                          nch = len(chunks)
                          # diag(r) term: psum += diag(sc_in/sc_sum) @ src
                          dg_t = dpool.tile([pb, pb], dt.float16, tag="dg")
                          if not no_dve:
                              nc.vector.tensor_scalar(
                                  out=dg_t[:], in0=ident_t[:],
                                  scalar1=r_t[:, b:b + 1], scalar2=None,
                                  op0=Alu.mult)
                          # fused one-hot build, FUSE chunks per DVE instruction
                          for f0 in range(0, nch, FUSE):
                              f1 = min(f0 + FUSE, nch)
                              nf = f1 - f0
                              dc = chunks[f0][1]
                              m_t = mpool.tile([pb, FUSE * pb], dt.float16, tag="m")
                              if not no_dve:
                                  nc.vector.tensor_tensor(
                                      out=m_t[:, :nf * pb].rearrange(
                                          "p (c e) -> p c e", e=pb),
                                      in0=iota_f[:, :nf * pb].rearrange(
                                          "p (c e) -> p c e", e=pb),
                                      in1=dst_t[:, dc:dc + nf].to_broadcast(
                                          [pb, nf, pb]),
                                      op=Alu.is_equal)
                              if no_pe:
                                  continue
                              for j in range(nf):
                                  ci = f0 + j
                                  gcol = chunks[ci][0]
                                  nc.tensor.matmul(
                                      out=psum_t[:], lhsT=m_t[:, j * pb:(j + 1) * pb],
                                      rhs=gt[:, (gcol - g0) * pb:(gcol - g0) * pb + D],
                                      start=(ci == 0), stop=False)
                          if not no_pe:
                              nc.tensor.matmul(
                                  out=psum_t[:], lhsT=dg_t[:],
                                  rhs=src_res[:, b * pb:b * pb + D],
                                  start=False, stop=True)
                          if no_vec:
                              continue
                          # dst = sc_sum * psum  (ACT: PSUM read + per-
                          # partition scale, keeps DVE free of PSUM waits)
                          if tail == "act":
                              nc.scalar.activation(
                                  out=dst_res[:, b * pb:b * pb + D],
                                  in_=psum_t[:],
                                  func=mybir.ActivationFunctionType.Copy,
                                  scale=sc_sum[:, b:b + 1])
                          else:
                              nc.vector.tensor_scalar(
                                  out=dst_res[:, b * pb:b * pb + D],
                                  in0=psum_t[:], scalar1=sc_sum[:, b:b + 1],
                                  scalar2=None, op0=Alu.mult)
                  if it == 0:
                      if stripe:
                          for i in range(nq):
                              zp = z_shard1[i]
                              nc.sync.dma_start(
                                  zp[:].rearrange("(p b) j -> p b j", p=st),
                                  z_res[1][st * i:st * (i + 1), :]
                                  .rearrange("p (b j) -> p b j", j=pb))
                              if no_cc:
                                  nc.gpsimd.dma_start(
                                      z_full1[i][0:sh // nq, :], zp[:])
                              else:
                                  nc.gpsimd.collective_compute(
                                      "AllGather", Alu.bypass,
                                      replica_groups=rg,
                                      ins=[zp[:]], outs=[z_full1[i][:]])
                      else:
                          nc.sync.dma_start(
                              z_shard1[:].rearrange("(p b) j -> p b j", p=pb),
                              z_res[1][:].rearrange("p (b j) -> p b j", j=pb))
                          if no_cc:
                              nc.gpsimd.dma_start(z_full1[0:sh, :], z_shard1[:])
                          else:
                              nc.gpsimd.collective_compute(
                                  "AllGather", Alu.bypass, replica_groups=rg,
                                  ins=[z_shard1[:]], outs=[z_full1[:]])
                  else:
                      nc.sync.dma_start(
                          u_dram[:].rearrange("(p b) j -> p b j", p=pb),
                          u_res[:].rearrange("p (b j) -> p b j", j=pb))

              # out = u @ W + b   (transposed reload of u gives lhsT)
              ut = res.tile([pb, sh], dt.float16, tag="zr0")
              nc.sync.dma_start(out=ut[:], in_=u_dram[:], transpose=True)
              out_res = work.tile([pb, nb * D], dt.float32, tag="gt")
              for i in range(nb):
                  psj = ps.tile([pb, D], dt.float32, tag="ps")
                  nc.tensor.matmul(out=psj[:], lhsT=ut[0:D, i * pb:(i + 1) * pb],
                                   rhs=w_t[:], start=True, stop=True)
                  nc.vector.tensor_tensor(
                      out=out_res[:, i * D:(i + 1) * D], in0=psj[:],
                      in1=bias_t[:], op=Alu.add)
              nc.sync.dma_start(
                  out_ext.rearrange("(p i) j -> p i j", p=pb),
                  out_res[:].rearrange("p (i j) -> p i j", j=D))

    nc.compile()
    return nc


# ---------------------------------------------------------------- unpermute
def unpermute_out(results, cfg, node_of_slot=None):
    ncores, pb, sh, nb, npad = (cfg["NCORES"], cfg["PB"], cfg["SH"], cfg["NB"],
                                cfg["NPAD"])
    if node_of_slot is None:
        node_of_slot = np.arange(npad, dtype=np.int64)
    out = np.empty((npad, D), np.float32)
    rp = np.arange(sh)
    p = rp // nb            # out_sh row r' = p*NB + i ...
    i = rp % nb
    r = i * pb + p          # ... holds u-flat row r = i*128 + p
    p2 = r // nb            # u-flat row r = p2*NB + b2  (partition major)
    b2 = r % nb
    slot_loc = b2 * pb + p2
    for c in range(ncores):
        out[node_of_slot[c * sh + slot_loc]] = results[c]["out_sh"][rp]
    return out[:cfg["N"]]


# ------------------------------------------------------------------- entry
def kernel(**inputs):
    x = np.asarray(inputs["x"])
    edge_index = np.asarray(inputs["edge_index"])
    W = np.asarray(inputs["W"])
    b = np.asarray(inputs["b"])

    _, _, _, _, run_bass_kernel_spmd = _bass_modules()
    cfg = _finish_cfg(DEFAULT_CFG)

    percore, layout = host_prep(x, edge_index, cfg)
    in_maps = make_in_maps(percore, W, b, cfg)
    key = layout["Cbq"].tobytes()
    if key not in _CACHE:
        _CACHE[key] = build_program(layout, nqueues=4)
    nc = _CACHE[key]

    res = run_bass_kernel_spmd(nc, in_maps, core_ids=list(range(cfg["NCORES"])))
    return unpermute_out(res.results, cfg, layout["node_of_slot"])

